# revision 2
# baseline (speedup 1.0000x reference)
"""Multi-head causal attention (B=4, L=2048, D=1024, H=16) on 8 trn2 cores.

Sharding: core c = (batch b = c//2) x (head-group hg = c%2, 8 heads each).
Each core computes, for its batch and its 8 heads:
  qkv projection -> qT,kT ([dim, L] layout) and v ([L, dim] layout)
  causal attention (S^T layout, softmax over the partition dim via a
  ones-row appended to v: AV matmul also produces the softmax denominator)
  output projection partial: o_heads @ Wproj[rows of those heads]
Host sums the two partials per batch and adds b_proj.

All matmuls run in fp32r (TF32) at bf16 speed; accumulation is fp32.
"""

import numpy as np

import concourse.bacc as bacc
import concourse.tile as tile
import concourse.mybir as mybir
from concourse.bass_utils import run_bass_kernel_spmd

B = 4
L = 2048
D = 1024
NH = 16
HD = 64
HPC = 8  # heads per core
DH = HPC * HD  # 512
NCORES = 8
CHUNK = 512  # Lq chunk
NCHUNKS = L // CHUNK  # 4
KT = D // 128  # 8 contraction tiles for the projections
F32 = mybir.dt.float32
F32R = mybir.dt.float32r
BIG_NEG = -1e9

_cache = {}


def _build():
    nc = bacc.Bacc("TRN2", dynamic_dma_scratch_size=2048)
    xT = nc.dram_tensor("xT", [D, L], F32, kind="ExternalInput")
    wqkv = nc.dram_tensor("wqkv", [D, 3 * DH], F32, kind="ExternalInput")
    bqkv = nc.dram_tensor("bqkv", [3 * DH], F32, kind="ExternalInput")
    wproj = nc.dram_tensor("wproj", [HD, HPC, D], F32, kind="ExternalInput")
    out = nc.dram_tensor("out", [L, D], F32, kind="ExternalOutput")

    wqkv_v = wqkv.rearrange("(k p) c -> p k c", p=128)  # [128, 8, 1536]
    xT_v = xT.rearrange("(k p) l -> p k l", p=128)  # [128, 8, 2048]

    with tile.TileContext(nc) as tc:
        with (
            nc.allow_low_precision(reason="fp32r (tf32) matmul pipeline"),
            tc.tile_pool(name="const", bufs=1) as const,
            tc.tile_pool(name="persist", bufs=1) as persist,
            tc.tile_pool(name="wqk_p", bufs=3) as wqk_p,
            tc.tile_pool(name="wv_p", bufs=1) as wv_p,
            tc.tile_pool(name="xc_p", bufs=1) as xc_p,
            tc.tile_pool(name="qt_p", bufs=2) as qt_p,
            tc.tile_pool(name="ot_p", bufs=1) as ot_p,
            tc.tile_pool(name="e_p", bufs=3) as e_p,
            tc.tile_pool(name="small_p", bufs=2) as small_p,
            tc.tile_pool(name="oev_p", bufs=2) as oev_p,
            tc.tile_pool(name="ps_big", bufs=2, space="PSUM") as ps_big,
            tc.tile_pool(name="ps_s", bufs=3, space="PSUM") as ps_s,
            tc.tile_pool(name="ps_av", bufs=2, space="PSUM") as ps_av,
            tc.tile_pool(name="ps_bc", bufs=1, space="PSUM") as ps_bc,
        ):
            # ---- constants / persistent state ----
            # biases: qk per m-tile [128, 8]; v per head [64, 8]
            bqk_t = const.tile([128, 8], F32)
            nc.sync.dma_start(
                out=bqk_t, in_=bqkv[0:DH * 2].rearrange("(m p) -> p m", p=128)
            )
            bv_t = const.tile([64, 8], F32)
            nc.sync.dma_start(
                out=bv_t, in_=bqkv[DH * 2 : DH * 3].rearrange("(h p) -> p h", p=64)
            )
            ones8 = const.tile([128, 8, 1], F32)
            nc.vector.memset(ones8, 1.0)
            ones_f = const.tile([128, 128], F32)
            nc.vector.memset(ones_f, 1.0)
            ones_r = const.tile([65, 128], F32R)
            nc.scalar.copy(ones_r[64:65, :], ones_f[64:65, :])
            # causal masks for the 4 diagonal tiles of a 512-wide q chunk:
            # valid iff 128*t + kp <= qf
            masks = const.tile([128, 4, CHUNK], F32)
            for t in range(4):
                nc.gpsimd.memset(masks[:, t, :], 0.0)
                nc.gpsimd.affine_select(
                    out=masks[:, t, :],
                    in_=masks[:, t, :],
                    compare_op=mybir.AluOpType.is_ge,
                    fill=BIG_NEG,
                    base=-128 * t,
                    pattern=[[1, CHUNK]],
                    channel_multiplier=-1,
                )
            # whole projection weight for phase 3: [64, 8, 1024] fp32r
            wproj_t = persist.tile([64, HPC, D], F32R)
            nc.sync.dma_start(out=wproj_t, in_=wproj[:].bitcast(F32R))
            # kT for all 8 heads, all L: [128, 4, 2048] (2 heads per tile)
            kt_t = persist.tile([128, 4, L], F32R)
            # v natural + ones column: [128, 16, 8, 65]
            v_t = persist.tile([128, L // 128, HPC, HD + 1], F32R)

            for c in range(NCHUNKS):
                c0 = c * CHUNK
                # ---- phase 1: projections for this chunk ----
                xc = xc_p.tile([128, KT, CHUNK], F32R)
                nc.sync.dma_start(
                    out=xc, in_=xT_v[:, :, c0 : c0 + CHUNK].bitcast(F32R)
                )
                qt = qt_p.tile([128, 4, CHUNK], F32R)
                for m in range(8):
                    wm = wqk_p.tile([128, KT, 128], F32R, tag="wqk")
                    nc.sync.dma_start(
                        out=wm,
                        in_=wqkv_v[:, :, m * 128 : (m + 1) * 128].bitcast(F32R),
                    )
                    ps = ps_big.tile([128, CHUNK], F32, tag="big")
                    for k in range(KT):
                        nc.tensor.matmul(
                            ps,
                            lhsT=wm[:, k, :],
                            rhs=xc[:, k, :],
                            start=(k == 0),
                            stop=(k == KT - 1),
                        )
                    if m < 4:
                        # q: (ps * 0.125) + bias, bias pre-scaled on host
                        nc.vector.tensor_scalar(
                            qt[:, m, :],
                            ps,
                            0.125,
                            bqk_t[:, m : m + 1],
                            mybir.AluOpType.mult,
                            mybir.AluOpType.add,
                        )
                    else:
                        nc.vector.tensor_scalar_add(
                            kt_t[:, m - 4, c0 : c0 + CHUNK],
                            ps,
                            bqk_t[:, m : m + 1],
                        )
                wv = wv_p.tile([128, KT, DH], F32R)
                nc.sync.dma_start(
                    out=wv, in_=wqkv_v[:, :, 2 * DH : 3 * DH].bitcast(F32R)
                )
                for t in range(4):
                    j = 4 * c + t
                    ps = ps_big.tile([128, DH], F32, tag="big")
                    for k in range(KT):
                        nc.tensor.matmul(
                            ps,
                            lhsT=xc[:, k, t * 128 : (t + 1) * 128],
                            rhs=wv[:, k, :],
                            start=(k == 0),
                            stop=(k == KT - 1),
                        )
                    nc.scalar.copy(
                        v_t[:, j, :, :HD], ps.rearrange("p (h d) -> p h d", d=HD)
                    )
                    nc.scalar.copy(v_t[:, j, :, HD : HD + 1], ones8)

                # ---- phase 2: attention for this chunk, per head ----
                ot = ot_p.tile([64, HPC, CHUNK], F32R)
                nj = 4 * c + 4  # causal: k tiles 0 .. 4c+3
                for h in range(HPC):
                    po = (h % 2) * 64
                    mt = h // 2
                    av = ps_av.tile([128, CHUNK], F32, tag="av")
                    for j in range(nj):
                        s = ps_s.tile([128, CHUNK], F32, tag="s")
                        nc.tensor.matmul(
                            s,
                            lhsT=kt_t[po : po + 64, mt, j * 128 : (j + 1) * 128],
                            rhs=qt[po : po + 64, mt, :],
                            start=True,
                            stop=True,
                        )
                        if j >= 4 * c:
                            nc.vector.tensor_add(
                                out=s, in0=s, in1=masks[:, j - 4 * c, :]
                            )
                        e = e_p.tile([128, CHUNK], F32R, tag="e")
                        nc.scalar.activation(
                            e, s, mybir.ActivationFunctionType.Exp
                        )
                        nc.tensor.matmul(
                            av[: HD + 1, :],
                            lhsT=v_t[:, j, h, :],
                            rhs=e,
                            start=(j == 0),
                            stop=(j == nj - 1),
                        )
                    rsum = small_p.tile([65, CHUNK], F32R, tag="rsum")
                    nc.vector.reciprocal(rsum[64:65, :], av[64:65, :])
                    bc = ps_bc.tile([128, CHUNK], F32)
                    nc.tensor.matmul(
                        bc,
                        lhsT=ones_r[64:65, :],
                        rhs=rsum[64:65, :],
                        start=True,
                        stop=True,
                    )
                    bc_sb = small_p.tile([64, CHUNK], F32, tag="bc")
                    nc.scalar.copy(bc_sb, bc[:64, :])
                    tmp = small_p.tile([64, CHUNK], F32, tag="tmp")
                    nc.vector.tensor_mul(tmp, av[:64, :], bc_sb)
                    nc.vector.tensor_scalar_add(
                        ot[:, h, :], tmp, bv_t[:, h : h + 1]
                    )

                # ---- phase 3: output projection for this chunk ----
                for m4 in range(4):
                    for n in range(2):
                        ps = ps_big.tile([128, CHUNK], F32, tag="big")
                        for h in range(HPC):
                            nc.tensor.matmul(
                                ps,
                                lhsT=ot[:, h, m4 * 128 : (m4 + 1) * 128],
                                rhs=wproj_t[:, h, n * CHUNK : (n + 1) * CHUNK],
                                start=(h == 0),
                                stop=(h == HPC - 1),
                            )
                        o_sb = oev_p.tile([128, CHUNK], F32, tag="oev")
                        nc.scalar.copy(o_sb, ps)
                        nc.sync.dma_start(
                            out=out[
                                c0 + m4 * 128 : c0 + (m4 + 1) * 128,
                                n * CHUNK : (n + 1) * CHUNK,
                            ],
                            in_=o_sb,
                        )

    nc.compile()
    return nc


def _get_nc():
    if "nc" not in _cache:
        _cache["nc"] = _build()
    return _cache["nc"]


def kernel(x, W_attn, b_attn, W_proj, b_proj):
    x = np.asarray(x, dtype=np.float32)
    W_attn = np.asarray(W_attn, dtype=np.float32)
    b_attn = np.asarray(b_attn, dtype=np.float32)
    W_proj = np.asarray(W_proj, dtype=np.float32)
    b_proj = np.asarray(b_proj, dtype=np.float32)

    nc = _get_nc()

    xTs = [np.ascontiguousarray(x[b].T) for b in range(B)]
    w4 = W_attn.reshape(D, 3, NH, HD)
    b4 = b_attn.reshape(3, NH, HD)
    in_maps = []
    for c in range(NCORES):
        b = c // 2
        hg = c % 2
        hsel = slice(hg * HPC, (hg + 1) * HPC)
        wqkv = np.ascontiguousarray(w4[:, :, hsel, :].reshape(D, 3 * DH))
        bq = b4[:, hsel, :].reshape(3 * DH).copy()
        bq[:DH] *= 0.125  # fold the q scale into the bias (device scales q*0.125)
        wproj = np.ascontiguousarray(
            W_proj[hg * DH : (hg + 1) * DH].reshape(HPC, HD, D).transpose(1, 0, 2)
        )
        in_maps.append(
            {"xT": xTs[b], "wqkv": wqkv, "bqkv": bq, "wproj": wproj}
        )

    results = run_bass_kernel_spmd(nc, in_maps, core_ids=list(range(NCORES))).results

    out = np.empty((B, L, D), dtype=np.float32)
    for b in range(B):
        out[b] = results[2 * b]["out"] + results[2 * b + 1]["out"] + b_proj
    return out
